# revision 36
# baseline (speedup 1.0000x reference)
"""Single-head causal self-attention (B=4, T=4096, C=1024, HS=64) on 8 TRN2 cores.

Sharding: core = 2*b + h; the two cores of batch b split the 8 query blocks
(512 rows each) in a load-balanced interleave (h=0 -> {0,3,4,7}, h=1 ->
{1,2,5,6}; 80 causal context chunks each).

The SPMD program is identical on every core; per-core differences are pure
data. Each core's context x[b] is PERMUTED host-side at 512-block granularity
so that the core's own query blocks sit at fixed program positions 0,2,4,6,
while every slot's causal context prefix is covered by the first 2(j+1)
permuted blocks. Causal-mask thresholds (per-core int32 data) absorb the
permutation; the mask ramp is built on-device with iota.

Dataflow per core, wavefront over 16 input granules of 256 columns:
  A1: [K^T|V^T] = [Wk|Wv]^T @ xt granule   (PSUM [128,512] per 512-block)
      V^T -> PE-transpose -> V natural [128k, 64], ones col appended
  A2 (even 512-blocks): Q^T for slot j from the block's own granules
  attention pairs (j, p) drained slot-major as soon as ready:
    C: S^T pair [128k, 2x512q] = K^T.T @ Q^T   (bf16, PSUM 2 banks)
    E: et = exp(0.125 * S^T) (ScalarE, one op per pair), mask last 4 pairs
    D: O[128q, 65] += et_chunk_qslice.T @ [V|1]_chunk  (65-wide moving side)
  finalize per q-tile: rec = 1/O[:,64]; out = O[:,0:64]*rec; DMA out per q-tile
"""

import numpy as np
import ml_dtypes

B, T, C, HS = 4, 4096, 1024, 64
QH = T // 2            # queries per core
NSLOT = 4
NCH = [8, 16, 24, 32]  # uniform context chunks (of 128) per slot
CCH = C // 128
BLOCKS = [[0, 3, 4, 7], [1, 2, 5, 6]]  # own query blocks per half
# permuted context layout: own blocks at positions 0,2,4,6; prefix-coverage
# of each slot's causal context holds for both halves
PERM = [[0, 1, 3, 2, 4, 5, 7, 6], [1, 0, 2, 3, 5, 4, 6, 7]]

_compiled = None

# emission order: A(pos) = projection/copy body; B(pos) = C+exp batch
# sched[pos]; F(pos) = deferred D/mask/finalize batch dplan[pos]
SEQPLAN = [("A", 0), ("A", 1), ("B", 1), ("A", 2), ("A", 3), ("B", 3),
           ("A", 4), ("A", 5), ("A", 6), ("A", 7), ("B", 5), ("F", 5),
           ("B", 7), ("A", 8), ("A", 9), ("B", 9), ("F", 9),
           ("A", 10), ("A", 11), ("B", 11), ("F", 11),
           ("A", 12), ("A", 13), ("B", 13), ("F", 13),
           ("A", 14), ("A", 15), ("B", 15), ("F", 15)]


# granule issue order: Q-block granule pairs (tb even) pulled forward so
# every slot's exp stream starts as early as possible
GORDER = [0, 1, 4, 5, 2, 3, 8, 9, 6, 7, 10, 11, 12, 13, 14, 15]


def _attn_schedule():
    """Returns (ce_stream, d_stream): ce_stream = (pos_gate, j, p) in Act
    order; d_stream = (pos_gate, j, p, fin_after) deferred D work. The
    emitter weaves small chunks of both behind each A(pos) body so the Act
    engine is fed continuously while stalled C matmuls (psC WAR, wait-queue
    depth 4) never block later PE work."""
    done_pos = {}
    for pos, g in enumerate(GORDER):
        if g % 2 == 1:
            done_pos[g // 2] = pos
    ready = {}
    for j in range(NSLOT):
        for p in range(NCH[j] // 2):
            ready[(j, p)] = max(done_pos[p // 2], done_pos[2 * j])
    ce = []
    for j in range(NSLOT):
        for p in range(NCH[j] // 2):
            ce.append((ready[(j, p)], j, p))
    ce.sort(key=lambda t: (t[0], t[1], t[2]))
    # tail: end the exp stream on unmasked pairs (3,10),(3,11)
    ce.remove((13, 3, 10))
    ce.remove((13, 3, 11))
    ce += [(15, 3, 10), (15, 3, 11)]
    # fins strictly ordered: ot runs with a single PSUM bank (bufs=1)
    dplan = [
        (5, 0, [0, 1, 2, 3], True),
        (9, 1, [0, 1, 4, 5, 2, 3, 6, 7], True),
        (13, 2, [0, 1, 4, 5, 2, 3, 6, 7, 8, 9, 10, 11], True),
        (13, 3, [0, 1, 4, 5, 8, 9, 2, 3, 6, 7], False),
        (15, 3, [12, 13, 14, 15, 10, 11], True),
    ]
    dstream = []
    emitted = [[] for _ in range(NSLOT)]
    cepos = {(j, p): g for g, j, p in ce}
    for pos, j, ps, fin in dplan:
        for i, p in enumerate(ps):
            assert cepos[(j, p)] <= pos, (j, p, pos)
            emitted[j].append(p)
            dstream.append((pos, j, p, fin and i == len(ps) - 1))
    for j in range(NSLOT):
        assert sorted(emitted[j]) == list(range(NCH[j] // 2)), j
    return ce, dstream


def _build_program():
    import concourse.bass as bass
    import concourse.mybir as mybir
    import concourse.tile as tile
    from concourse import bacc
    from concourse.masks import make_identity
    from contextlib import ExitStack

    f32 = mybir.dt.float32
    bf16 = mybir.dt.bfloat16
    i32 = mybir.dt.int32

    nc = bacc.Bacc("TRN2", target_bir_lowering=False, debug=False, num_devices=8)

    xt_d = nc.dram_tensor("xt", [C, T], bf16, kind="ExternalInput").ap()
    wkv8_d = nc.dram_tensor("wkv8", [128, CCH * 128], bf16,
                            kind="ExternalInput").ap()
    wq8_d = nc.dram_tensor("wq8", [128, CCH * HS], bf16,
                           kind="ExternalInput").ap()
    thr2_d = nc.dram_tensor("thr2", [128, 16], f32, kind="ExternalInput").ap()
    out_d = nc.dram_tensor("out", [QH, HS + 1], f32,
                       kind="ExternalOutput").ap()

    ce_stream, d_stream = _attn_schedule()

    with tile.TileContext(nc) as tc, ExitStack() as ctx:
        consts = ctx.enter_context(tc.tile_pool(name="consts", bufs=1))
        epool = ctx.enter_context(tc.tile_pool(name="epool", bufs=3))
        mpool = ctx.enter_context(tc.tile_pool(name="mpool", bufs=2))

        xt = consts.tile([128, CCH, T], bf16)
        wkv = consts.tile([128, CCH, 128], bf16)
        wq = consts.tile([128, CCH, HS], bf16)
        ramp2 = consts.tile([128, 2, 512], i32)
        thr2 = consts.tile([128, 16], f32)
        id_bf = consts.tile([64, 64], bf16)
        zsc = consts.tile([64, 512], bf16)
        kTv = consts.tile([64, T], bf16)
        qTv = consts.tile([64, QH], bf16)
        vp = consts.tile([128, T // 128, HS + 1], bf16)  # [V | ones]
        outs = consts.tile([128, QH // 128, HS + 1], f32)

        # DMA order tuned for the critical path: wkv -> granule 0 -> wq ->
        # granule 1 -> thr2 -> remaining granules
        xt_r = xt_d.rearrange("(a p) t -> p a t", p=128)

        def xtg(g):
            sl = slice(g * 256, g * 256 + 256)
            nc.sync.dma_start(out=xt[:, :, sl], in_=xt_r[:, :, sl])

        nc.sync.dma_start(out=wkv,
                          in_=wkv8_d.rearrange("p (a m) -> p a m", a=CCH))
        xtg(GORDER[0])
        nc.sync.dma_start(out=wq,
                          in_=wq8_d.rearrange("p (a m) -> p a m", a=CCH))
        xtg(GORDER[1])
        nc.sync.dma_start(out=thr2, in_=thr2_d)
        for pos in range(2, 16):
            xtg(GORDER[pos])

        nc.vector.memset(zsc, 0.0)
        make_identity(nc, id_bf)
        nc.vector.memset(vp[:, :, HS], 1.0)
        # ramp2[p, d, q] = q - 128*d, built on-device (no DMA)
        nc.gpsimd.iota(ramp2, pattern=[[-128, 2], [1, 512]],
                       base=0, channel_multiplier=0)
        # mask pair tiles; generation is spread through the weave (2 per
        # position) so GPSIMD interleaves them with the vp copies
        mk = [consts.tile([128, 2, 512], bf16, name=f"mk_{i}") for i in range(16)]

        def gen_masks(pos):
            for i in (2 * pos, 2 * pos + 1):
                if i < 16:
                    nc.gpsimd.tensor_scalar(
                        mk[i], ramp2, thr2[:, i:i + 1], None,
                        op0=mybir.AluOpType.is_ge)

        with tc.tile_pool(name="psA", bufs=2, space="PSUM") as psA, \
             tc.tile_pool(name="psC", bufs=2, space="PSUM") as psC, \
             tc.tile_pool(name="psO", bufs=1, space="PSUM") as psO:
            ot = [None] * NSLOT
            ets = {}

            # PE warmup: zero-matmul chain pins pe_busy_start early so the
            # p-state clock is at full speed when the first projection lands
            for w in range(2):
                pw = psA.tile([64, 512], f32, tag="pa", name=f"warm_{w}")
                nc.tensor.matmul(pw, zsc[0:64, 0:64], zsc,
                                 start=True, stop=True)

            def emit_ce(j, p):
                pc = psC.tile([128, 1024], f32, tag="pc", name=f"pc_{j}_{p}")
                qsl = slice(j * 512, j * 512 + 512)
                for d in range(2):
                    kk = 2 * p + d
                    osl = slice(d * 512, d * 512 + 512)
                    ksl = slice(kk * 128, kk * 128 + 128)
                    nc.tensor.matmul(pc[:, osl], kTv[:, ksl], qTv[:, qsl],
                                     start=True, stop=True)
                et = epool.tile([128, 2, 512], bf16, tag="et", bufs=28,
                                name=f"et_{j}_{p}")
                nc.scalar.activation(et, pc,
                                     mybir.ActivationFunctionType.Exp,
                                     scale=0.125)
                ets[(j, p)] = et

            def emit_d(j, p, fin_after):
                if ot[j] is None:
                    ot[j] = psO.tile([128, 4, HS + 1], f32, tag="ot",
                                     name=f"ot_{j}")
                et = ets.pop((j, p))
                m = p - (NCH[j] // 2 - 4)
                if m >= 0:
                    # mask applied here (exp long done: no DVE convoy)
                    nc.vector.tensor_mul(et, et, mk[4 * j + m])
                first = p == 0
                for d in range(2):
                    kk = 2 * p + d
                    for qs in range(4):
                        # start zeroes the whole PSUM bank: set only on the
                        # slot's first emitted matmul; one stop on the last
                        nc.tensor.matmul(
                            ot[j][:, qs, :],
                            et[:, d, qs * 128:qs * 128 + 128],
                            vp[:, kk, :],
                            start=(first and d == 0 and qs == 0),
                            stop=(fin_after and d == 1 and qs == 3))
                if fin_after:
                    # unnormalized [O | denom] copied out wholesale; the
                    # softmax division happens host-side
                    nc.vector.tensor_copy(
                        outs[:, 4 * j:4 * j + 4, :], ot[j])
                    nc.sync.dma_start(
                        out=out_d.rearrange("(q p) h -> p q h", p=128)[
                            :, 4 * j:4 * j + 4, :],
                        in_=outs[:, 4 * j:4 * j + 4, :])

            pa_cur = pq_cur = None

            def emit_A(pos):
                nonlocal pa_cur, pq_cur
                g = GORDER[pos]
                tb, half = g // 2, g % 2
                sl = slice(g * 256, g * 256 + 256)
                hsl = slice(half * 256, half * 256 + 256)
                own = tb % 2 == 0
                # A2 first: the first C of a slot needs only qTv (its kT
                # chunks are old), so Q's projection is the critical path
                if own:
                    j = tb // 2
                    if half == 0:
                        pq_cur = psA.tile([64, 512], f32, tag="pa",
                                          name=f"pq_{j}")
                    for ci in range(CCH):
                        nc.tensor.matmul(pq_cur[:, hsl], wq[:, ci, :],
                                         xt[:, ci, sl],
                                         start=(ci == 0 and half == 0),
                                         stop=(ci == CCH - 1 and half == 1))
                if half == 0:
                    pa_cur = psA.tile([128, 512], f32, tag="pa",
                                      name=f"pa_{tb}")
                for ci in range(CCH):
                    nc.tensor.matmul(pa_cur[:, hsl], wkv[:, ci, :],
                                     xt[:, ci, sl],
                                     start=(ci == 0 and half == 0),
                                     stop=(ci == CCH - 1 and half == 1))
                if half == 1:
                    if own:
                        j = tb // 2
                        nc.vector.tensor_copy(
                            qTv[:, j * 512:j * 512 + 512], pq_cur[0:64, :])
                    bsl = slice(tb * 512, tb * 512 + 512)
                    nc.vector.tensor_copy(kTv[:, bsl], pa_cur[0:64, :])
                    vts = epool.tile([64, 512], bf16, tag="vts",
                                     name=f"vts_{tb}")
                    nc.vector.tensor_copy(vts, pa_cur[64:128, :])
                    vtp4 = psA.tile([128, 4, HS], bf16, tag="vtp",
                                    bufs=1, name=f"vtp4_{tb}")
                    for blk in range(4):
                        nc.tensor.matmul(
                            vtp4[:, blk, :], vts[:, blk * 128:blk * 128 + 128],
                            id_bf, is_transpose=True,
                            start=(blk == 0), stop=(blk == 3))
                    for blk in range(4):
                        kk = tb * 4 + blk
                        nc.vector.tensor_copy(vp[:, kk, 0:HS], vtp4[:, blk, :])

            # weave: after each A(pos) body, alternate 2-CE and 2-D
            # chunks whose gates have opened; stalled Cs then never clog the
            # PE wait queue ahead of projections or D work
            ci_, di_ = 0, 0
            for pos in range(16):
                gen_masks(pos)
                emit_A(pos)
                while True:
                    did = False
                    for _ in range(2):
                        if ci_ < len(ce_stream) and ce_stream[ci_][0] <= pos:
                            _, j, p = ce_stream[ci_]
                            emit_ce(j, p)
                            ci_ += 1
                            did = True
                    for _ in range(2):
                        if di_ < len(d_stream) and d_stream[di_][0] <= pos \
                                and (d_stream[di_][1], d_stream[di_][2]) in ets:
                            _, j, p, fin = d_stream[di_]
                            emit_d(j, p, fin)
                            di_ += 1
                            did = True
                    if not did:
                        break
            assert ci_ == len(ce_stream) and di_ == len(d_stream)

    nc.compile()
    return nc


def _prep_inputs(x, Wq, Wk, Wv):
    bf = ml_dtypes.bfloat16
    wkv = np.concatenate([Wk, Wv], axis=1)               # [C, 128]
    wkv8 = wkv.reshape(CCH, 128, 128).transpose(1, 0, 2).reshape(128, -1)
    wq8 = Wq.reshape(CCH, 128, HS).transpose(1, 0, 2).reshape(128, -1)
    wkv8 = np.ascontiguousarray(wkv8).astype(bf)
    wq8 = np.ascontiguousarray(wq8).astype(bf)
    p = np.arange(128, dtype=np.int64)
    in_maps = []
    for core in range(8):
        b, h = core // 2, core % 2
        perm = PERM[h]
        xt = np.concatenate(
            [x[b, g * 512:(g + 1) * 512] for g in perm], axis=0
        ).T.astype(bf)
        thr2 = np.zeros((128, 16), np.float32)
        for j in range(NSLOT):
            g = perm[2 * j]
            for pm in range(4):
                kk0 = NCH[j] - 8 + 2 * pm
                base0 = 512 * perm[kk0 // 4] + 128 * (kk0 % 4)
                thr2[:, 4 * j + pm] = base0 + p - 512 * g
        in_maps.append({
            "xt": np.ascontiguousarray(xt),
            "wkv8": wkv8, "wq8": wq8, "thr2": thr2,
        })
    return in_maps


def kernel(x, Wq, Wk, Wv):
    from concourse.bass_utils import run_bass_kernel_spmd

    global _compiled
    if _compiled is None:
        _compiled = _build_program()
    nc = _compiled

    in_maps = _prep_inputs(
        np.asarray(x, np.float32), np.asarray(Wq, np.float32),
        np.asarray(Wk, np.float32), np.asarray(Wv, np.float32),
    )
    res = run_bass_kernel_spmd(nc, in_maps, list(range(8)))
    out = np.empty((B, T, HS), np.float32)
    for core in range(8):
        b, h = core // 2, core % 2
        perm = PERM[h]
        o = res.results[core]["out"]
        o = o[:, 0:HS] / o[:, HS:HS + 1]
        for j in range(NSLOT):
            g = perm[2 * j]
            out[b, g * 512:(g + 1) * 512] = o[j * 512:(j + 1) * 512]
    return out


if __name__ == "__main__":
    rng = np.random.default_rng(0)
    x = rng.standard_normal((B, T, C), dtype=np.float32)
    s = 1 / np.sqrt(C)
    Wq = rng.standard_normal((C, HS), dtype=np.float32) * s
    Wk = rng.standard_normal((C, HS), dtype=np.float32) * s
    Wv = rng.standard_normal((C, HS), dtype=np.float32) * s
    o = kernel(x=x, Wq=Wq, Wk=Wk, Wv=Wv)
    print(o.shape, o.dtype, np.abs(o).mean())


# revision 37
# speedup vs baseline: 1.0067x; 1.0067x over previous
"""Single-head causal self-attention (B=4, T=4096, C=1024, HS=64) on 8 TRN2 cores.

Sharding: core = 2*b + h; the two cores of batch b split the 8 query blocks
(512 rows each) in a load-balanced interleave (h=0 -> {0,3,4,7}, h=1 ->
{1,2,5,6}; 80 causal context chunks each).

The SPMD program is identical on every core; per-core differences are pure
data. Each core's context x[b] is PERMUTED host-side at 512-block granularity
so that the core's own query blocks sit at fixed program positions 0,2,4,6,
while every slot's causal context prefix is covered by the first 2(j+1)
permuted blocks. Causal-mask thresholds (per-core int32 data) absorb the
permutation; the mask ramp is built on-device with iota.

Dataflow per core, wavefront over 16 input granules of 256 columns:
  A1: [K^T|V^T] = [Wk|Wv]^T @ xt granule   (PSUM [128,512] per 512-block)
      V^T -> PE-transpose -> V natural [128k, 64], ones col appended
  A2 (even 512-blocks): Q^T for slot j from the block's own granules
  attention pairs (j, p) drained slot-major as soon as ready:
    C: S^T pair [128k, 2x512q] = K^T.T @ Q^T   (bf16, PSUM 2 banks)
    E: et = exp(0.125 * S^T) (ScalarE, one op per pair), mask last 4 pairs
    D: O[128q, 65] += et_chunk_qslice.T @ [V|1]_chunk  (65-wide moving side)
  finalize per q-tile: rec = 1/O[:,64]; out = O[:,0:64]*rec; DMA out per q-tile
"""

import numpy as np
import ml_dtypes

B, T, C, HS = 4, 4096, 1024, 64
QH = T // 2            # queries per core
NSLOT = 4
NCH = [8, 16, 24, 32]  # uniform context chunks (of 128) per slot
CCH = C // 128
BLOCKS = [[0, 3, 4, 7], [1, 2, 5, 6]]  # own query blocks per half
# permuted context layout: own blocks at positions 0,2,4,6; prefix-coverage
# of each slot's causal context holds for both halves
PERM = [[0, 1, 3, 2, 4, 5, 7, 6], [1, 0, 2, 3, 5, 4, 6, 7]]

_compiled = None

# emission order: A(pos) = projection/copy body; B(pos) = C+exp batch
# sched[pos]; F(pos) = deferred D/mask/finalize batch dplan[pos]
SEQPLAN = [("A", 0), ("A", 1), ("B", 1), ("A", 2), ("A", 3), ("B", 3),
           ("A", 4), ("A", 5), ("A", 6), ("A", 7), ("B", 5), ("F", 5),
           ("B", 7), ("A", 8), ("A", 9), ("B", 9), ("F", 9),
           ("A", 10), ("A", 11), ("B", 11), ("F", 11),
           ("A", 12), ("A", 13), ("B", 13), ("F", 13),
           ("A", 14), ("A", 15), ("B", 15), ("F", 15)]


# granule issue order: Q-block granule pairs (tb even) pulled forward so
# every slot's exp stream starts as early as possible
GORDER = [0, 1, 4, 5, 2, 3, 8, 9, 6, 7, 10, 11, 12, 13, 14, 15]


def _attn_schedule():
    """Returns (ce_stream, d_stream): ce_stream = (pos_gate, j, p) in Act
    order; d_stream = (pos_gate, j, p, fin_after) deferred D work. The
    emitter weaves small chunks of both behind each A(pos) body so the Act
    engine is fed continuously while stalled C matmuls (psC WAR, wait-queue
    depth 4) never block later PE work."""
    done_pos = {}
    for pos, g in enumerate(GORDER):
        if g % 2 == 1:
            done_pos[g // 2] = pos
    ready = {}
    for j in range(NSLOT):
        for p in range(NCH[j] // 2):
            ready[(j, p)] = max(done_pos[p // 2], done_pos[2 * j])
    ce = []
    for j in range(NSLOT):
        for p in range(NCH[j] // 2):
            ce.append((ready[(j, p)], j, p))
    ce.sort(key=lambda t: (t[0], t[1], t[2]))
    # tail: end the exp stream on unmasked pairs (3,10),(3,11)
    ce.remove((13, 3, 10))
    ce.remove((13, 3, 11))
    ce += [(15, 3, 10), (15, 3, 11)]
    # fins strictly ordered: ot runs with a single PSUM bank (bufs=1)
    dplan = [
        (5, 0, [0, 1, 2, 3], True),
        (9, 1, [0, 1, 4, 5, 2, 3, 6, 7], True),
        (13, 2, [0, 1, 4, 5, 2, 3, 6, 7, 8, 9, 10, 11], True),
        (13, 3, [0, 1, 4, 5, 8, 9, 2, 3, 6, 7], False),
        (15, 3, [12, 13, 14, 15, 10, 11], True),
    ]
    dstream = []
    emitted = [[] for _ in range(NSLOT)]
    cepos = {(j, p): g for g, j, p in ce}
    for pos, j, ps, fin in dplan:
        for i, p in enumerate(ps):
            assert cepos[(j, p)] <= pos, (j, p, pos)
            emitted[j].append(p)
            dstream.append((pos, j, p, fin and i == len(ps) - 1))
    for j in range(NSLOT):
        assert sorted(emitted[j]) == list(range(NCH[j] // 2)), j
    return ce, dstream


def _build_program():
    import concourse.bass as bass
    import concourse.mybir as mybir
    import concourse.tile as tile
    from concourse import bacc
    from concourse.masks import make_identity
    from contextlib import ExitStack

    f32 = mybir.dt.float32
    bf16 = mybir.dt.bfloat16
    i32 = mybir.dt.int32

    nc = bacc.Bacc("TRN2", target_bir_lowering=False, debug=False, num_devices=8)

    xt_d = nc.dram_tensor("xt", [C, T], bf16, kind="ExternalInput").ap()
    wkv8_d = nc.dram_tensor("wkv8", [128, CCH * 128], bf16,
                            kind="ExternalInput").ap()
    wq8_d = nc.dram_tensor("wq8", [128, CCH * HS], bf16,
                           kind="ExternalInput").ap()
    thr2_d = nc.dram_tensor("thr2", [128, 16], f32, kind="ExternalInput").ap()
    out_d = nc.dram_tensor("out", [QH, HS + 1], f32,
                       kind="ExternalOutput").ap()

    ce_stream, d_stream = _attn_schedule()

    with tile.TileContext(nc) as tc, ExitStack() as ctx:
        consts = ctx.enter_context(tc.tile_pool(name="consts", bufs=1))
        epool = ctx.enter_context(tc.tile_pool(name="epool", bufs=3))
        mpool = ctx.enter_context(tc.tile_pool(name="mpool", bufs=2))

        xt = consts.tile([128, CCH, T], bf16)
        wkv = consts.tile([128, CCH, 128], bf16)
        wq = consts.tile([128, CCH, HS], bf16)
        ramp2 = consts.tile([128, 2, 512], i32)
        thr2 = consts.tile([128, 16], f32)
        id_bf = consts.tile([64, 64], bf16)
        zsc = consts.tile([64, 512], bf16)
        kTv = consts.tile([64, T], bf16)
        qTv = consts.tile([64, QH], bf16)
        vp = consts.tile([128, T // 128, HS + 1], bf16)  # [V | ones]
        outs = consts.tile([128, QH // 128, HS + 1], f32)

        # DMA order tuned for the critical path: wkv -> granule 0 -> wq ->
        # granule 1 -> thr2 -> remaining granules
        xt_r = xt_d.rearrange("(a p) t -> p a t", p=128)

        def xtg(g):
            sl = slice(g * 256, g * 256 + 256)
            nc.sync.dma_start(out=xt[:, :, sl], in_=xt_r[:, :, sl])

        nc.sync.dma_start(out=wkv,
                          in_=wkv8_d.rearrange("p (a m) -> p a m", a=CCH))
        xtg(GORDER[0])
        nc.sync.dma_start(out=wq,
                          in_=wq8_d.rearrange("p (a m) -> p a m", a=CCH))
        xtg(GORDER[1])
        nc.sync.dma_start(out=thr2, in_=thr2_d)
        for pos in range(2, 16):
            xtg(GORDER[pos])

        nc.vector.memset(zsc, 0.0)
        make_identity(nc, id_bf)
        nc.vector.memset(vp[:, :, HS], 1.0)
        # ramp2[p, d, q] = q - 128*d, built on-device (no DMA)
        nc.gpsimd.iota(ramp2, pattern=[[-128, 2], [1, 512]],
                       base=0, channel_multiplier=0)
        # mask pair tiles; generation is spread through the weave (2 per
        # position) so GPSIMD interleaves them with the vp copies
        mk = [consts.tile([128, 2, 512], bf16, name=f"mk_{i}") for i in range(16)]

        def gen_masks(pos):
            for i in (2 * pos, 2 * pos + 1):
                if i < 16:
                    nc.gpsimd.tensor_scalar(
                        mk[i], ramp2, thr2[:, i:i + 1], None,
                        op0=mybir.AluOpType.is_ge)

        with tc.tile_pool(name="psA", bufs=2, space="PSUM") as psA, \
             tc.tile_pool(name="psC", bufs=2, space="PSUM") as psC, \
             tc.tile_pool(name="psO", bufs=1, space="PSUM") as psO:
            ot = [None] * NSLOT
            ets = {}

            # PE warmup: zero-matmul chain pins pe_busy_start early so the
            # p-state clock is at full speed when the first projection lands
            for w in range(2):
                pw = psA.tile([64, 512], f32, tag="pa", name=f"warm_{w}")
                nc.tensor.matmul(pw, zsc[0:64, 0:64], zsc,
                                 start=True, stop=True)

            def emit_ce(j, p):
                pc = psC.tile([128, 1024], f32, tag="pc", name=f"pc_{j}_{p}")
                qsl = slice(j * 512, j * 512 + 512)
                for d in range(2):
                    kk = 2 * p + d
                    osl = slice(d * 512, d * 512 + 512)
                    ksl = slice(kk * 128, kk * 128 + 128)
                    nc.tensor.matmul(pc[:, osl], kTv[:, ksl], qTv[:, qsl],
                                     start=True, stop=True)
                et = epool.tile([128, 2, 512], bf16, tag="et", bufs=28,
                                name=f"et_{j}_{p}")
                nc.scalar.activation(et, pc,
                                     mybir.ActivationFunctionType.Exp,
                                     scale=0.125)
                ets[(j, p)] = et

            def emit_d(j, p, fin_after):
                if ot[j] is None:
                    ot[j] = psO.tile([128, 4, HS + 1], f32, tag="ot",
                                     name=f"ot_{j}")
                et = ets.pop((j, p))
                m = p - (NCH[j] // 2 - 4)
                if m >= 0:
                    # mask applied here (exp long done: no DVE convoy)
                    nc.vector.tensor_mul(et, et, mk[4 * j + m])
                first = p == 0
                for d in range(2):
                    kk = 2 * p + d
                    for qs in range(4):
                        # start zeroes the whole PSUM bank: set only on the
                        # slot's first emitted matmul; one stop on the last
                        nc.tensor.matmul(
                            ot[j][:, qs, :],
                            et[:, d, qs * 128:qs * 128 + 128],
                            vp[:, kk, :],
                            start=(first and d == 0 and qs == 0),
                            stop=(fin_after and d == 1 and qs == 3))
                if fin_after:
                    # unnormalized [O | denom] copied out wholesale; the
                    # softmax division happens host-side
                    nc.vector.tensor_copy(
                        outs[:, 4 * j:4 * j + 4, :], ot[j])
                    nc.sync.dma_start(
                        out=out_d.rearrange("(q p) h -> p q h", p=128)[
                            :, 4 * j:4 * j + 4, :],
                        in_=outs[:, 4 * j:4 * j + 4, :])

            pa_cur = pq_cur = None

            def emit_A(pos):
                nonlocal pa_cur, pq_cur
                g = GORDER[pos]
                tb, half = g // 2, g % 2
                sl = slice(g * 256, g * 256 + 256)
                hsl = slice(half * 256, half * 256 + 256)
                own = tb % 2 == 0
                # A2 first: the first C of a slot needs only qTv (its kT
                # chunks are old), so Q's projection is the critical path
                if own:
                    j = tb // 2
                    if half == 0:
                        pq_cur = psA.tile([64, 512], f32, tag="pa",
                                          name=f"pq_{j}")
                    for ci in range(CCH):
                        nc.tensor.matmul(pq_cur[:, hsl], wq[:, ci, :],
                                         xt[:, ci, sl],
                                         start=(ci == 0 and half == 0),
                                         stop=(ci == CCH - 1 and half == 1))
                if half == 0:
                    pa_cur = psA.tile([128, 512], f32, tag="pa",
                                      name=f"pa_{tb}")
                for ci in range(CCH):
                    nc.tensor.matmul(pa_cur[:, hsl], wkv[:, ci, :],
                                     xt[:, ci, sl],
                                     start=(ci == 0 and half == 0),
                                     stop=(ci == CCH - 1 and half == 1))
                if half == 1:
                    if own:
                        j = tb // 2
                        nc.vector.tensor_copy(
                            qTv[:, j * 512:j * 512 + 512], pq_cur[0:64, :])
                    bsl = slice(tb * 512, tb * 512 + 512)
                    nc.vector.tensor_copy(kTv[:, bsl], pa_cur[0:64, :])
                    vts = epool.tile([64, 512], bf16, tag="vts",
                                     name=f"vts_{tb}")
                    nc.vector.tensor_copy(vts, pa_cur[64:128, :])
                    vtp4 = psA.tile([128, 4, HS], bf16, tag="vtp",
                                    bufs=1, name=f"vtp4_{tb}")
                    for blk in range(4):
                        nc.tensor.matmul(
                            vtp4[:, blk, :], vts[:, blk * 128:blk * 128 + 128],
                            id_bf, is_transpose=True,
                            start=(blk == 0), stop=(blk == 3))
                    nc.vector.tensor_copy(
                        vp[:, 4 * tb:4 * tb + 4, 0:HS], vtp4)

            # weave: after each A(pos) body, alternate 2-CE and 2-D
            # chunks whose gates have opened; stalled Cs then never clog the
            # PE wait queue ahead of projections or D work
            ci_, di_ = 0, 0
            for pos in range(16):
                gen_masks(pos)
                emit_A(pos)
                while True:
                    did = False
                    for _ in range(2):
                        if ci_ < len(ce_stream) and ce_stream[ci_][0] <= pos:
                            _, j, p = ce_stream[ci_]
                            emit_ce(j, p)
                            ci_ += 1
                            did = True
                    for _ in range(2):
                        if di_ < len(d_stream) and d_stream[di_][0] <= pos \
                                and (d_stream[di_][1], d_stream[di_][2]) in ets:
                            _, j, p, fin = d_stream[di_]
                            emit_d(j, p, fin)
                            di_ += 1
                            did = True
                    if not did:
                        break
            assert ci_ == len(ce_stream) and di_ == len(d_stream)

    nc.compile()
    return nc


def _prep_inputs(x, Wq, Wk, Wv):
    bf = ml_dtypes.bfloat16
    wkv = np.concatenate([Wk, Wv], axis=1)               # [C, 128]
    wkv8 = wkv.reshape(CCH, 128, 128).transpose(1, 0, 2).reshape(128, -1)
    wq8 = Wq.reshape(CCH, 128, HS).transpose(1, 0, 2).reshape(128, -1)
    wkv8 = np.ascontiguousarray(wkv8).astype(bf)
    wq8 = np.ascontiguousarray(wq8).astype(bf)
    p = np.arange(128, dtype=np.int64)
    in_maps = []
    for core in range(8):
        b, h = core // 2, core % 2
        perm = PERM[h]
        xt = np.concatenate(
            [x[b, g * 512:(g + 1) * 512] for g in perm], axis=0
        ).T.astype(bf)
        thr2 = np.zeros((128, 16), np.float32)
        for j in range(NSLOT):
            g = perm[2 * j]
            for pm in range(4):
                kk0 = NCH[j] - 8 + 2 * pm
                base0 = 512 * perm[kk0 // 4] + 128 * (kk0 % 4)
                thr2[:, 4 * j + pm] = base0 + p - 512 * g
        in_maps.append({
            "xt": np.ascontiguousarray(xt),
            "wkv8": wkv8, "wq8": wq8, "thr2": thr2,
        })
    return in_maps


def kernel(x, Wq, Wk, Wv):
    from concourse.bass_utils import run_bass_kernel_spmd

    global _compiled
    if _compiled is None:
        _compiled = _build_program()
    nc = _compiled

    in_maps = _prep_inputs(
        np.asarray(x, np.float32), np.asarray(Wq, np.float32),
        np.asarray(Wk, np.float32), np.asarray(Wv, np.float32),
    )
    res = run_bass_kernel_spmd(nc, in_maps, list(range(8)))
    out = np.empty((B, T, HS), np.float32)
    for core in range(8):
        b, h = core // 2, core % 2
        perm = PERM[h]
        o = res.results[core]["out"]
        o = o[:, 0:HS] / o[:, HS:HS + 1]
        for j in range(NSLOT):
            g = perm[2 * j]
            out[b, g * 512:(g + 1) * 512] = o[j * 512:(j + 1) * 512]
    return out


if __name__ == "__main__":
    rng = np.random.default_rng(0)
    x = rng.standard_normal((B, T, C), dtype=np.float32)
    s = 1 / np.sqrt(C)
    Wq = rng.standard_normal((C, HS), dtype=np.float32) * s
    Wk = rng.standard_normal((C, HS), dtype=np.float32) * s
    Wv = rng.standard_normal((C, HS), dtype=np.float32) * s
    o = kernel(x=x, Wq=Wq, Wk=Wk, Wv=Wv)
    print(o.shape, o.dtype, np.abs(o).mean())
